# revision 7
# baseline (speedup 1.0000x reference)
"""Trainium2 Bass kernel for DSS-GIN conv (gnn_message_passing).

Strategy (8 NeuronCores, B=128 subgraphs sharded 16/core):
  - h = MLP_t(X) computed per-core in transposed space via PE matmuls (fp32r).
  - Pooled node branch: per-core partial max over local subgraphs, AllReduce(max)
    across cores (split into 4 column groups, pipelined behind stage A), then
    nodex = MLP_n(xmax) replicated on every core.
  - Message passing ret1 + broadcast nodex2 folded into ONE dense matmul:
      out[b] = S^T @ (h[b] + nodex)   where S[j,k] = #edges j->k  (built on host
    from edge_index, streamed from DRAM as 128x128 tiles).
  - All matmuls in float32r (full PE rate at free>=256, ~12-bit mantissa).
"""
import sys
sys.path.insert(0, '/opt/trn_rl_repo')

import numpy as np


def _ensure_ntff_hook_module():
    """Provide antenv.axon_hooks if the image lacks it (needed only when
    BASS_TRACE=1 requests NTFF profiling through run_bass_kernel_spmd)."""
    try:
        import antenv.axon_hooks  # noqa: F401
        return
    except Exception:
        pass
    import contextlib, ctypes, os, types

    mod = types.ModuleType("antenv.axon_hooks")
    state = {"hook": None, "tried": False}
    so_path = "/opt/axon/libaxon_pjrt.so"

    def _make_hook(path):
        lib = ctypes.CDLL(path)
        if not hasattr(lib, "axon_start_nrt_profile"):
            return None
        lib.axon_start_nrt_profile.argtypes = [
            ctypes.POINTER(ctypes.c_int64), ctypes.c_size_t]
        lib.axon_start_nrt_profile.restype = ctypes.c_int64
        lib.axon_stop_nrt_profile.argtypes = [ctypes.c_char_p]
        lib.axon_stop_nrt_profile.restype = ctypes.c_int64

        @contextlib.contextmanager
        def _hook(output_dir, device_ids):
            import jax
            jax.devices()
            if device_ids:
                ids = (ctypes.c_int64 * len(device_ids))(*device_ids)
                rc = lib.axon_start_nrt_profile(ids, len(device_ids))
            else:
                rc = lib.axon_start_nrt_profile(None, 0)
            if rc != 0:
                raise RuntimeError(f"axon_start_nrt_profile rc={rc}")
            try:
                yield
            finally:
                n = lib.axon_stop_nrt_profile(str(output_dir).encode())
                if n < 0:
                    raise RuntimeError(f"axon_stop_nrt_profile rc={n}")
                print(f"profile: {n} file(s) written to {output_dir}")

        return _hook

    def get_axon_ntff_profile_hook():
        if state["hook"] is None and not state["tried"]:
            state["tried"] = True
            if os.path.exists(so_path):
                try:
                    state["hook"] = _make_hook(so_path)
                except Exception:
                    state["hook"] = None
        return state["hook"]

    def set_axon_ntff_profile_hook(hook):
        state["hook"] = hook
        state["tried"] = True

    mod.get_axon_ntff_profile_hook = get_axon_ntff_profile_hook
    mod.set_axon_ntff_profile_hook = set_axon_ntff_profile_hook
    sys.modules["antenv.axon_hooks"] = mod


_ensure_ntff_hook_module()

NCORES = 8
B, N, D, E = 128, 2048, 64, 32768
BL = B // NCORES          # 16 subgraphs per core
NT = N // 128             # 16 node tiles
NCHUNK = 512              # bn-chunk: 4 node tiles for one subgraph
NG = N // NCHUNK          # 4 chunks per subgraph

_BUILD_CACHE = {}
LAST_RESULTS = None


def _build():
    if "nc" in _BUILD_CACHE:
        return _BUILD_CACHE["nc"]
    import concourse.bacc as bacc
    import concourse.tile as tile
    from concourse import mybir
    dt = mybir.dt
    f32, f32r = dt.float32, dt.float32r
    Relu = mybir.ActivationFunctionType.Relu
    Alu = mybir.AluOpType

    nc = bacc.Bacc("TRN2", target_bir_lowering=False, debug=False)

    Xc = nc.dram_tensor("Xc", [BL, N, D], f32, kind="ExternalInput").ap()
    St = nc.dram_tensor("St", [NT, NT, 128, 128], f32r, kind="ExternalInput").ap()
    W1t = nc.dram_tensor("W1t", [D, D], f32, kind="ExternalInput").ap()
    B1t = nc.dram_tensor("B1t", [D, 1], f32, kind="ExternalInput").ap()
    W2t = nc.dram_tensor("W2t", [D, D], f32, kind="ExternalInput").ap()
    B2t = nc.dram_tensor("B2t", [D, 1], f32, kind="ExternalInput").ap()
    W1n = nc.dram_tensor("W1n", [D, D], f32, kind="ExternalInput").ap()
    B1n = nc.dram_tensor("B1n", [D, 1], f32, kind="ExternalInput").ap()
    W2n = nc.dram_tensor("W2n", [D, D], f32, kind="ExternalInput").ap()
    B2n = nc.dram_tensor("B2n", [D, 1], f32, kind="ExternalInput").ap()
    W1DD = nc.dram_tensor("W1DD", [128, 128], f32, kind="ExternalInput").ap()
    W2DD = nc.dram_tensor("W2DD", [128, 128], f32, kind="ExternalInput").ap()
    B1DD = nc.dram_tensor("B1DD", [128, 1], f32, kind="ExternalInput").ap()
    B2DD = nc.dram_tensor("B2DD", [128, 1], f32, kind="ExternalInput").ap()
    Ident = nc.dram_tensor("Ident", [128, 128], f32, kind="ExternalInput").ap()
    Out = nc.dram_tensor("Out", [BL, N, D], f32, kind="ExternalOutput").ap()

    with tile.TileContext(nc) as tc:
        with tc.tile_pool(name="const", bufs=1) as constp, \
             tc.tile_pool(name="resident", bufs=1) as resp, \
             tc.tile_pool(name="dram", bufs=1, space="DRAM") as dram:

            # ---- constants ----
            ident = constp.tile([128, 128], f32)
            nc.sync.dma_start(ident[:], Ident[:])
            ident_r = constp.tile([64, 64], f32r)
            nc.vector.tensor_copy(ident_r[:], ident[:64, :64])
            ident_r128 = constp.tile([128, 128], f32r)
            nc.vector.tensor_copy(ident_r128[:], ident[:])

            wdd_f32 = constp.tile([128, 2 * 128], f32)
            nc.sync.dma_start(wdd_f32[:, 0:128], W1DD[:])
            nc.sync.dma_start(wdd_f32[:, 128:256], W2DD[:])
            wdd_r = constp.tile([128, 2 * 128], f32r)
            nc.vector.tensor_copy(wdd_r[:], wdd_f32[:])
            w1dd, w2dd = wdd_r[:, 0:128], wdd_r[:, 128:256]
            bdd = constp.tile([128, 2], f32)
            nc.sync.dma_start(bdd[:, 0:1], B1DD[:])
            nc.sync.dma_start(bdd[:, 1:2], B2DD[:])
            b1dd, b2dd = bdd[:, 0:1], bdd[:, 1:2]

            w_f32 = constp.tile([D, 4 * D], f32)
            nc.sync.dma_start(w_f32[:, 0 * D:1 * D], W1t[:])
            nc.sync.dma_start(w_f32[:, 1 * D:2 * D], W2t[:])
            nc.sync.dma_start(w_f32[:, 2 * D:3 * D], W1n[:])
            nc.sync.dma_start(w_f32[:, 3 * D:4 * D], W2n[:])
            w_r = constp.tile([D, 4 * D], f32r)
            nc.vector.tensor_copy(w_r[:], w_f32[:])
            w1t, w2t = w_r[:, 0 * D:1 * D], w_r[:, 1 * D:2 * D]
            w1n, w2n = w_r[:, 2 * D:3 * D], w_r[:, 3 * D:4 * D]

            biases = constp.tile([D, 4], f32)
            nc.sync.dma_start(biases[:, 0:1], B1t[:])
            nc.sync.dma_start(biases[:, 1:2], B2t[:])
            nc.sync.dma_start(biases[:, 2:3], B1n[:])
            nc.sync.dma_start(biases[:, 3:4], B2n[:])
            b1t, b2t = biases[:, 0:1], biases[:, 1:2]
            b1n, b2n = biases[:, 2:3], biases[:, 3:4]

            # ---- resident tensors ----
            # h layout: [128 part, jt(16), b(16), d(64)] = [128, 16384]
            h_all = resp.tile([128, NT, BL, D], f32r)
            xpt2 = resp.tile([128, N], f32r)   # partial max (even b rows 0-63, odd b rows 64-127)
            xpt = resp.tile([64, N], f32r)     # combined partial max
            xmax = resp.tile([64, N], f32r)    # global max (post allreduce)
            nodex = resp.tile([128, NT, D], f32)

            # ============ stage A + B: MLP_t, pooled branch (per g) ============
            with tc.tile_pool(name="xn", bufs=4) as xnp, \
                 tc.tile_pool(name="xt", bufs=3) as xtp, \
                 tc.tile_pool(name="mid", bufs=3) as midp, \
                 tc.tile_pool(name="htt", bufs=3) as http, \
                 tc.tile_pool(name="psA", bufs=2, space="PSUM") as psA:

                for g in range(NG):
                    gs = slice(g * NCHUNK, (g + 1) * NCHUNK)
                    for p in range(BL // 2):
                        b0 = 2 * p
                        # two subgraphs packed along the free/partition dims
                        xn = xnp.tile([128, 4, 2, D], f32, tag="xn")
                        for j in range(2):
                            nc.sync.dma_start(
                                xn[:, :, j, :],
                                Xc[b0 + j, g * NCHUNK:(g + 1) * NCHUNK, :].rearrange(
                                    "(t p) d -> p t d", p=128),
                            )
                        # transpose -> [128 (b,d), 512 n]
                        tp = psA.tile([128, NCHUNK], f32, tag="tp")
                        for t in range(4):
                            nc.tensor.transpose(
                                tp[:, t * 128:(t + 1) * 128],
                                xn[:, t, :, :].rearrange("p b d -> p (b d)"),
                                ident[:])
                        xt = xtp.tile([128, NCHUNK], f32r, tag="xt")
                        nc.vector.tensor_copy(xt[:], tp[:])
                        # partial max for pooled branch (even/odd halves)
                        if p == 0:
                            nc.vector.tensor_copy(xpt2[:, gs], xt[:])
                        else:
                            nc.vector.tensor_tensor(
                                xpt2[:, gs], xpt2[:, gs], xt[:], Alu.max)
                        # L1/L2 with block-diagonal weights (2 subgraphs at once)
                        l1p = psA.tile([128, NCHUNK], f32, tag="l1p")
                        nc.tensor.matmul(l1p[:], w1dd, xt[:], start=True, stop=True)
                        mid = midp.tile([128, NCHUNK], f32r, tag="mid")
                        nc.scalar.activation(mid[:], l1p[:], Relu, bias=b1dd)
                        l2p = psA.tile([128, NCHUNK], f32, tag="l2p")
                        nc.tensor.matmul(l2p[:], w2dd, mid[:], start=True, stop=True)
                        htt = http.tile([128, NCHUNK], f32r, tag="htt")
                        nc.scalar.activation(htt[:], l2p[:], Relu, bias=b2dd)
                        # transpose back: [128 n, (2b x 64d)] slabs into h_all
                        htp = psA.tile([128, 2, D], f32r, tag="htp")
                        for t in range(4):
                            jt = 4 * g + t
                            nc.tensor.transpose(
                                htp[:].rearrange("p b d -> p (b d)"),
                                htt[:, t * 128:(t + 1) * 128], ident_r128[:])
                            nc.vector.tensor_copy(
                                h_all[:, jt, b0:b0 + 2, :], htp[:])

                    # combine even/odd partial maxes (DMA shifts partitions 64-127 down)
                    xodd = xtp.tile([64, NCHUNK], f32r, tag="xodd")
                    nc.sync.dma_start(xodd[:], xpt2[64:128, gs])
                    nc.vector.tensor_tensor(
                        xpt[:, gs], xpt2[:64, gs], xodd[:], Alu.max)

                    # -- pooled branch for this column group --
                    cin = dram.tile([64, NCHUNK], f32, tag="cin", bufs=4)
                    cout = dram.tile([64, NCHUNK], f32, tag="cout", bufs=4)
                    nc.sync.dma_start(cin[:], xpt[:, gs].bitcast(f32))
                    nc.gpsimd.collective_compute(
                        "AllReduce",
                        Alu.max,
                        replica_groups=[list(range(NCORES))],
                        ins=[cin[:].opt()],
                        outs=[cout[:].opt()],
                    )
                    nc.sync.dma_start(xmax[:, gs].bitcast(f32), cout[:])

                    l1p = psA.tile([64, NCHUNK], f32, tag="l1p")
                    nc.tensor.matmul(l1p[:], w1n, xmax[:, gs], start=True, stop=True)
                    mid = midp.tile([64, NCHUNK], f32r, tag="mid")
                    nc.scalar.activation(mid[:], l1p[:], Relu, bias=b1n)
                    l2p = psA.tile([64, NCHUNK], f32, tag="l2p")
                    nc.tensor.matmul(l2p[:], w2n, mid[:], start=True, stop=True)
                    htt = http.tile([64, NCHUNK], f32r, tag="htt")
                    nc.scalar.activation(htt[:], l2p[:], Relu, bias=b2n)
                    htp = psA.tile([128, 4, D], f32r, tag="htp")
                    for t in range(4):
                        nc.tensor.transpose(
                            htp[:, t, :],
                            htt[:, t * 128:(t + 1) * 128], ident_r[:])
                    nc.vector.tensor_copy(nodex[:, 4 * g:4 * g + 4, :], htp[:])

                    # h' = h + nodex (broadcast over b) for this group's jt tiles
                    nc.vector.tensor_tensor(
                        h_all[:, 4 * g:4 * g + 4, :, :],
                        h_all[:, 4 * g:4 * g + 4, :, :],
                        nodex[:, 4 * g:4 * g + 4, None, :].broadcast_to(
                            (128, 4, BL, D)),
                        Alu.add)

            # =================== stage C: out[b] = S^T @ h' ===================
            with tc.tile_pool(name="ssl", bufs=4) as sslp, \
                 tc.tile_pool(name="osb", bufs=3) as osbp, \
                 tc.tile_pool(name="psC", bufs=3, space="PSUM") as psC:
                for kt in range(NT):
                    ssl = sslp.tile([128, NT, 128], f32r, tag="ssl")
                    nc.sync.dma_start(
                        ssl[:], St[kt].rearrange("jt p k -> p jt k"))
                    pc = psC.tile([128, BL * D], f32, tag="pc")
                    for jt in range(NT):
                        for half in range(2):
                            hs = slice(half * 512, (half + 1) * 512)
                            nc.tensor.matmul(
                                pc[:, hs],
                                ssl[:, jt, :],
                                h_all[:, jt, :, :].rearrange("p b d -> p (b d)")[:, hs],
                                start=(jt == 0), stop=(jt == NT - 1))
                    osb = osbp.tile([128, BL * D], f32, tag="osb")
                    nc.scalar.activation(osb[:], pc[:], Relu)
                    # one DMA for the whole ktile: DRAM side strided per (p, b)
                    nc.sync.dma_start(
                        Out[:, kt * 128:(kt + 1) * 128, :].rearrange(
                            "b p d -> p b d"),
                        osb[:].rearrange("p (b d) -> p b d", d=D))

    nc.compile()
    _BUILD_CACHE["nc"] = nc
    return nc


def kernel(X, edge_index, W1t, b1t, W2t, b2t, W1n, b1n, W2n, b2n):
    global LAST_RESULTS
    from concourse.bass_utils import run_bass_kernel_spmd

    nc = _build()

    X = np.ascontiguousarray(X, dtype=np.float32)
    # dense adjacency S[src, dst] = edge count, tiled [kt, jt, 128, 128]
    S = np.zeros((N, N), dtype=np.float32)
    np.add.at(S, (edge_index[0].astype(np.int64), edge_index[1].astype(np.int64)), 1.0)
    St = np.ascontiguousarray(
        S.reshape(NT, 128, NT, 128).transpose(2, 0, 1, 3))

    common = {
        "St": St,
        "W1t": np.ascontiguousarray(W1t, np.float32),
        "B1t": np.ascontiguousarray(b1t, np.float32).reshape(D, 1),
        "W2t": np.ascontiguousarray(W2t, np.float32),
        "B2t": np.ascontiguousarray(b2t, np.float32).reshape(D, 1),
        "W1n": np.ascontiguousarray(W1n, np.float32),
        "B1n": np.ascontiguousarray(b1n, np.float32).reshape(D, 1),
        "W2n": np.ascontiguousarray(W2n, np.float32),
        "B2n": np.ascontiguousarray(b2n, np.float32).reshape(D, 1),
        "W1DD": np.block([
            [np.asarray(W1t, np.float32), np.zeros((D, D), np.float32)],
            [np.zeros((D, D), np.float32), np.asarray(W1t, np.float32)]]),
        "W2DD": np.block([
            [np.asarray(W2t, np.float32), np.zeros((D, D), np.float32)],
            [np.zeros((D, D), np.float32), np.asarray(W2t, np.float32)]]),
        "B1DD": np.concatenate([np.asarray(b1t, np.float32).ravel()] * 2).reshape(128, 1),
        "B2DD": np.concatenate([np.asarray(b2t, np.float32).ravel()] * 2).reshape(128, 1),
        "Ident": np.eye(128, dtype=np.float32),
    }
    in_maps = [
        {"Xc": np.ascontiguousarray(X[c * BL:(c + 1) * BL]), **common}
        for c in range(NCORES)
    ]
    res = run_bass_kernel_spmd(nc, in_maps, list(range(NCORES)))
    LAST_RESULTS = res
    out = np.empty((B, N, D), dtype=np.float32)
    for c in range(NCORES):
        out[c * BL:(c + 1) * BL] = res.results[c]["Out"]
    return out
